# revision 24
# baseline (speedup 1.0000x reference)
"""Trainium2 Bass kernel for nn_AA_encoder (gnn_message_passing).

Data-parallel over sentences B=512 -> 64 per core on 8 NeuronCores.
Sparse-graph decomposition: each sentence has <=8 real node slots
(4 aspect + 4 clause); packed 8x8 attention per sentence plus analytic
reconstruction of the full 48x48 softmax / top-k / GCN.

Self-contained: host-side index preprocessing (numpy) + one SPMD Bass
program executed via run_bass_kernel_spmd on cores 0-7.
"""
import sys
import numpy as np

sys.path.insert(0, '/opt/trn_rl_repo')

import concourse.bass as bass
import concourse.bacc as bacc
import concourse.tile as tile
from concourse import mybir
from concourse.bass_utils import run_bass_kernel_spmd

B, L, T, D = 512, 48, 128, 300
NA = NC = 2048
H, KTOP = 4, 96
DK = D // H
NCORE = 8
BS = B // NCORE          # 64 sentences per core
A = C = 4                # aspects / clauses per sentence
R = A + C                # 8 real rows per sentence
NCAND = 80
F32 = mybir.dt.float32
AX = mybir.AxisListType
OP = mybir.AluOpType
AF = mybir.ActivationFunctionType

# ---------------------------------------------------------------- aux layout
# one packed [128, NAUX] constant tensor -> single DMA
_AUX_SPECS = [
    ('maskT', 128, 256), ('vcm', 128, 512), ('lennv', 128, 4),
    ('hm0', 128, 128), ('hm1', 128, 128), ('spreadm', 128, 256),
    ('oneh8', 128, 8), ('blk128', 128, 512),
    ('candwh', 128, 40), ('sumh', 128, 64),
    ('candw', 64, NCAND), ('candc', 128, NCAND), ('cmask', 64, NCAND),
    ('mv', 64, 64), ('dg64', 64, 64), ('mfrow', 64, R),
    ('linv', 64, 1), ('lennv64', 64, 1),
    ('wlmap', 64, 512), ('wlpick', 128, 32),
    ('rdmap', 64, 256), ('rdpick', 128, 4),
]
_AUX_OFF = {}
_off = 0
for _nm, _rows, _cols in _AUX_SPECS:
    _AUX_OFF[_nm] = (_rows, _off, _cols)
    _off += _cols
NAUX = _off

# layout constants: p = 32*qq + 8*ss + ii
_SPREADM = np.zeros((128, 256), np.float32)
_ONEH8 = np.zeros((128, 8), np.float32)
_BLK128 = np.zeros((128, 512), np.float32)
_SUMH = np.zeros((128, 64), np.float32)
_HM = np.zeros((2, 128, 128), np.float32)
_WLMAP = np.zeros((64, 512), np.float32)
_WLPICK = np.zeros((128, 32), np.float32)
_RDMAP = np.zeros((64, 256), np.float32)
_RDPICK = np.zeros((128, 4), np.float32)
for _p in range(128):
    _qq, _ss, _ii = _p // 32, (_p % 32) // 8, _p % 8
    _ONEH8[_p, _ii] = 1.0
    for _c4 in range(4):
        _s16g = 16*_c4 + 4*_qq + _ss
        _BLK128[_p, 128*_c4 + _s16g] = 1.0
        _BLK128[_p, 128*_c4 + 64 + _s16g] = 1.0
    _SUMH[_p, _p % 64] = 1.0
    for _j in range(8):
        _SPREADM[_p, 32*_ii + 8*_ss + _j] = 1.0
    _i16, _j8 = _p // 8, _p % 8
    for _kc in range(2):
        if _i16 < 16:
            _HM[_kc, _p, 64*_kc + 4*_i16: 64*_kc + 4*_i16 + 4] = 1.0
    for _mt in range(2):
        for _kc in range(2):
            _s = 32*_mt + 16*_kc + _i16
            _WLMAP[_s, 128*(2*_mt + _kc) + _p] = 1.0
    _WLPICK[_p, _j8::8] = 1.0          # [j == p%8] in each ct group
    for _mt in range(2):
        _RDMAP[32*_mt + _p // 4, 128*_mt + _p] = 1.0
    _RDPICK[_p, _p % 4] = 1.0


# ----------------------------------------------------------------- host prep
def _host_prep(inputs):
    ie = np.ascontiguousarray(np.asarray(inputs['input_embed'], np.float32))
    co = np.ascontiguousarray(np.asarray(inputs['clause_output'], np.float32))
    bm = np.asarray(inputs['batch_aa_mask'], np.float32)
    gl = np.asarray(inputs['aa_graph_length']).astype(np.int64)
    mAA = np.asarray(inputs['map_AA']).astype(np.int64)
    mAAi = np.asarray(inputs['map_AA_idx']).astype(np.int64)
    mAS = np.asarray(inputs['map_AS']).astype(np.int64)
    mASi = np.asarray(inputs['map_AS_idx']).astype(np.int64)
    for bname in ('bq', 'bk', 'bg'):
        assert np.all(np.asarray(inputs[bname]) == 0), f"{bname} != 0 unsupported"

    asp_rows = [[] for _ in range(B)]
    for n in range(NA):
        asp_rows[mAA[n]].append(n)
    cls_rows = [[] for _ in range(B)]
    for n in range(NC):
        cls_rows[mAS[n]].append(n)
    assert max(len(x) for x in asp_rows) <= A
    assert max(len(x) for x in cls_rows) <= C

    slots = np.full((B, R), -1, np.int64)
    maskTg = np.zeros((B, T, A), np.float32)
    clsg = np.zeros((B, C, D), np.float32)
    out_row = np.full((NC,), -1, np.int64)
    out_cidx = np.full((NC,), -1, np.int64)
    for b in range(B):
        cslots = set()
        for j, n in enumerate(cls_rows[b]):
            s = mASi[n]
            assert s not in cslots
            cslots.add(s)
            slots[b, A + j] = s
            clsg[b, j] = co[n]
            out_row[n] = b
            out_cidx[n] = j
        aslots = set()
        for j, n in enumerate(asp_rows[b]):
            s = mAAi[n]
            assert s not in aslots
            aslots.add(s)
            if s in cslots:
                continue
            slots[b, j] = s
            maskTg[b, :, j] = bm[n]

    lens = gl
    vr = (slots >= 0)
    vc = vr & (slots < lens[:, None])
    nv = vc.sum(1)

    cores = []
    for c in range(NCORE):
        s0 = c * BS
        sl_vc = vc[s0:s0+BS]
        nvc = nv[s0:s0+BS]
        ln = lens[s0:s0+BS]

        vcm = np.zeros((4, 128, 128), np.float32)
        lennv = np.zeros((4, 128, 1), np.float32)
        for s in range(BS):
            ch, i16 = s // 16, s % 16
            qq, ss = i16 // 4, i16 % 4
            vcj = sl_vc[s].astype(np.float32)
            for ii in range(R):
                p = 32*qq + 8*ss + ii
                lennv[ch, p, 0] = ln[s] - nvc[s]
                for h in range(H):
                    vcm[ch, p, 32*h + 8*ss: 32*h + 8*ss + 8] = vcj

        candw = np.zeros((BS, NCAND), np.float32)
        candc = np.zeros((BS, NCAND), np.float32)
        cmask = np.zeros((BS, NCAND), np.float32)
        mv = np.zeros((BS, 64), np.float32)
        dg64 = np.zeros((BS, 64), np.float32)
        mfrow = np.zeros((BS, R), np.float32)
        linv = (1.0 / ln[:, None]).astype(np.float32)
        lennv64 = (ln - nvc)[:, None].astype(np.float32)
        for s in range(BS):
            l, n_v = int(ln[s]), int(nvc[s])
            vcs = sl_vc[s]
            for i in range(R):
                for j in range(R):
                    if i != j and vcs[i] and vcs[j]:
                        candw[s, 8*i+j] = 1.0
                        cmask[s, 8*i+j] = 1.0
                        mv[s, 8*i+j] = 1.0
                if vcs[i]:
                    candw[s, 64+i] = l - n_v
                    cmask[s, 64+i] = 1.0 if l > n_v else 0.0
                    dg64[s, 8*i+i] = 1.0
                    mfrow[s, i] = 1.0
            candc[s, 72] = 1.0; candw[s, 72] = l; cmask[s, 72] = 1.0
            candc[s, 73] = 1.0/l
            candw[s, 73] = (l - n_v) * (l - 1)
            cmask[s, 73] = 1.0 if (l - n_v) > 0 and l > 1 else 0.0
            used = l + (l-n_v)*(l-1) + sum(
                (n_v-1) + (l-n_v) for i in range(R) if vcs[i])
            candw[s, 74] = L*L - used
            cmask[s, 74] = 1.0

        clsc = clsg[s0:s0+BS].reshape(BS*C, D)

        aux = np.zeros((128, NAUX), np.float32)
        parts = dict(
            maskT=np.ascontiguousarray(
                maskTg[s0:s0+BS].transpose(1, 0, 2).reshape(T, BS*A)),
            vcm=vcm.transpose(1, 0, 2).reshape(128, 512),
            lennv=lennv.transpose(1, 0, 2).reshape(128, 4),
            hm0=_HM[0], hm1=_HM[1], spreadm=_SPREADM, oneh8=_ONEH8,
            blk128=_BLK128, sumh=_SUMH,
            candwh=np.concatenate([candw[:, 0:40], candw[:, 40:80]], axis=0),
            candw=candw, candc=np.vstack([candc, candc]),
            cmask=cmask,
            mv=mv, dg64=dg64, mfrow=mfrow, linv=linv, lennv64=lennv64,
            wlmap=_WLMAP, wlpick=_WLPICK, rdmap=_RDMAP, rdpick=_RDPICK,
        )
        for nm, arr in parts.items():
            rows, off, cols = _AUX_OFF[nm]
            assert arr.shape == (rows, cols), (nm, arr.shape)
            aux[:rows, off:off+cols] = arr

        cores.append(dict(
            emb=ie[s0:s0+BS],
            clsT=np.ascontiguousarray(clsc.T),             # [300, 256]
            aux=aux,
        ))
    post = dict(out_row=out_row, out_cidx=out_cidx, lens=lens)
    return cores, post


# -------------------------------------------------------------- bass program
def _make_nc():
    nc = bacc.Bacc("TRN2", target_bir_lowering=False, debug=False,
                   enable_asserts=False, num_devices=NCORE)
    io = {}
    specs = dict(
        emb=(BS, T, D), clsT=(D, BS*C), aux=(128, NAUX),
        WqT=(D, D), WkT=(D, D), WgT=(D, D),
    )
    for k, shp in specs.items():
        io[k] = nc.dram_tensor(k, list(shp), F32, kind="ExternalInput")
    out_d = nc.dram_tensor("out", [2*128, D], F32, kind="ExternalOutput")
    return nc, io, out_d


def build_nc():
    nc, io, out_d = _make_nc()
    with tile.TileContext(nc) as tc:
        _build_body(nc, tc, io, out_d)
    nc.finalize()
    return nc


def build_nc_debug(names):
    nc, io, out_d = _make_nc()
    dbg = {'names': set(names), 'dumps': []}
    with tile.TileContext(nc) as tc:
        _build_body(nc, tc, io, out_d, dbg=dbg)
        for nm, t in dbg['dumps']:
            od = nc.dram_tensor(f"dbg_{nm}", list(t.shape), F32,
                                kind="ExternalOutput")
            nc.sync.dma_start(out=od[tuple(slice(None) for _ in t.shape)],
                              in_=t)
    nc.finalize()
    return nc


def _build_body(nc, tc, io, out_d, dbg=None):
    import contextlib

    def _cap(name, t):
        if dbg is not None and name in dbg['names']:
            dbg['dumps'].append((name, t))

    ctx = contextlib.ExitStack()
    with ctx:
        singles = ctx.enter_context(tc.tile_pool(name="singles", bufs=1))
        embp = ctx.enter_context(tc.tile_pool(name="embp", bufs=8))
        work = ctx.enter_context(tc.tile_pool(name="work", bufs=4))
        ps = ctx.enter_context(tc.tile_pool(name="ps", bufs=8, space="PSUM"))

        # ---- resident constants: ONE aux DMA + 3 weight DMAs
        aux = singles.tile([128, NAUX], F32)
        nc.sync.dma_start(out=aux[:, :], in_=io['aux'][:, :])

        def ax(nm):
            rows, off, cols = _AUX_OFF[nm]
            return aux[0:rows, off:off+cols]

        wq = singles.tile([100, 3*D], F32, tag="w0", name="wq")
        wk = singles.tile([100, 3*D], F32, tag="w1", name="wk")
        wg = singles.tile([100, 3*D], F32, tag="w2", name="wg")
        for wtile, wio in ((wq, io['WqT']), (wk, io['WkT']), (wg, io['WgT'])):
            nc.sync.dma_start(
                out=wtile.rearrange("r (c d) -> r c d", c=3),
                in_=wio.rearrange("(c r) d -> r c d", c=3))

        maskT = ax('maskT')

        # ---- fused per-chunk pipeline: aa -> qk/g -> scores -> softmax
        featsT = [[singles.tile([100, 128], F32, tag=f"fT{dc}_{c}",
                                 name=f"fT{dc}_{c}") for c in range(4)]
                  for dc in range(3)]
        g = [singles.tile([128, D], F32, tag=f"g{c}", name=f"g{c}")
             for c in range(4)]
        qT = [[singles.tile([75, 128], F32, tag=f"qT{h}_{c}",
                             name=f"qT{h}_{c}") for c in range(4)]
              for h in range(H)]
        kT = [[singles.tile([75, 128], F32, tag=f"kT{h}_{c}",
                            name=f"kT{h}_{c}") for c in range(4)]
              for h in range(H)]
        ap8 = [work.tile([128, 32], F32, tag=f"ap8{c}", name=f"ap8{c}")
               for c in range(4)]
        fm = [work.tile([128, 1], F32, tag=f"fm{c}", name=f"fm{c}")
              for c in range(4)]
        sprd = [work.tile([128, 264], F32, tag=f"sprd{c}", name=f"sprd{c}")
                for c in range(4)]
        emb_r = io['emb']  # [64, 128, 300]
        sc = float(1.0 / np.sqrt(DK))
        for c in range(4):
            # --- aa-embed (transposed) via 4-sentence emb bursts
            for g4 in range(4):
                esb = embp.tile([T, 4*D], F32, tag="emb")
                nc.sync.dma_start(
                    out=esb.rearrange("t (s d) -> t s d", s=4),
                    in_=emb_r[16*c+4*g4:16*c+4*g4+4].rearrange(
                        "s t d -> t s d"))
                pst = [ps.tile([100, 32], F32, tag="ps",
                               name=f"pst{c}_{g4}_{i}") for i in range(3)]
                for sl in range(4):
                    s = 16*c + 4*g4 + sl
                    for dc in range(3):
                        nc.tensor.matmul(
                            pst[dc][:, 8*sl:8*sl+4],
                            lhsT=esb[:, 300*sl+100*dc:300*sl+100*dc+100],
                            rhs=maskT[:, 4*s:4*s+4],
                            start=True, stop=True, skip_group_check=True)
                for dc in range(3):
                    nc.scalar.activation(
                        out=featsT[dc][c][:, 32*g4:32*g4+32],
                        in_=pst[dc], func=AF.Copy)
            # clause columns (overwrite garbage at p=8i+4..8i+8)
            for dc in range(3):
                dst = featsT[dc][c].rearrange(
                    "d (i r) -> d i r", i=16)[:, :, 4:8]
                src = io['clsT'][100*dc:100*dc+100, 64*c:64*c+64].rearrange(
                    "d (i r) -> d i r", r=4)
                nc.gpsimd.dma_start(out=dst, in_=src)
            # --- q/k projections (head-major [75, 128]) and g row-major
            for h in range(H):
                pq = ps.tile([75, 128], F32, tag="ps", name=f"pq{c}_{h}")
                for k in range(3):
                    nc.tensor.matmul(
                        pq, lhsT=wq[:, k*D+75*h:k*D+75*h+75],
                        rhs=featsT[k][c],
                        start=(k == 0), stop=(k == 2))
                nc.scalar.activation(out=qT[h][c][:, :],
                                     in_=pq, func=AF.Copy, scale=sc)
                pk = ps.tile([75, 128], F32, tag="ps", name=f"pk{c}_{h}")
                for k in range(3):
                    nc.tensor.matmul(
                        pk, lhsT=wk[:, k*D+75*h:k*D+75*h+75],
                        rhs=featsT[k][c],
                        start=(k == 0), stop=(k == 2))
                nc.scalar.activation(out=kT[h][c][:, :],
                                     in_=pk, func=AF.Copy)
            pg = ps.tile([128, D], F32, tag="ps", name=f"pg{c}")
            for k in range(3):
                nc.tensor.matmul(
                    pg, lhsT=featsT[k][c],
                    rhs=wg[:, k*D:k*D+D], start=(k == 0), stop=(k == 2))
            nc.scalar.activation(out=g[c], in_=pg, func=AF.Copy)
            # --- scores
            vcm_c = ax('vcm')[:, 128*c:128*c+128]
            lennv_c = ax('lennv')[:, c:c+1]
            psc = ps.tile([128, 128], F32, tag="ps", name=f"psc{c}")
            for qq in range(4):
                q0 = 32*qq
                for h in range(H):
                    nc.tensor.matmul(
                        psc[32*qq:32*qq+32, 32*h:32*h+32],
                        lhsT=qT[h][c][:, q0:q0+32],
                        rhs=kT[h][c][:, q0:q0+32],
                        start=True, stop=True, skip_group_check=True,
                        tile_position=(0, 32*qq))
            # --- masked softmax reconstruction
            smb = work.tile([128, 128], F32, tag="smb")
            nc.vector.tensor_tensor(out=smb, in0=psc, in1=vcm_c, op=OP.mult)
            m4 = work.tile([128, 4], F32, tag="m4")
            nc.vector.tensor_reduce(out=m4, in_=smb.rearrange(
                "p (h j) -> p h j", h=4), axis=AX.X, op=OP.max)
            nc.vector.tensor_scalar_max(out=m4, in0=m4, scalar1=0.0)
            es = work.tile([128, 128], F32, tag="es")
            nc.vector.tensor_tensor(
                out=es.rearrange("p (h j) -> p h j", h=4),
                in0=smb.rearrange("p (h j) -> p h j", h=4),
                in1=m4.unsqueeze(2).to_broadcast((128, 4, 32)),
                op=OP.subtract)
            nc.scalar.activation(out=es, in_=es, func=AF.Exp)
            nc.vector.tensor_tensor(out=es, in0=es, in1=vcm_c, op=OP.mult)
            z4 = work.tile([128, 4], F32, tag="z4")
            nc.vector.tensor_reduce(out=z4, in_=es.rearrange(
                "p (h j) -> p h j", h=4), axis=AX.X, op=OP.add)
            em = work.tile([128, 4], F32, tag="em")
            nc.scalar.activation(out=em, in_=m4, func=AF.Exp, scale=-1.0)
            nc.vector.scalar_tensor_tensor(
                out=z4, in0=em, scalar=lennv_c, in1=z4,
                op0=OP.mult, op1=OP.add)
            rz = work.tile([128, 4], F32, tag="rz")
            nc.vector.reciprocal(out=rz, in_=z4)
            nc.vector.tensor_scalar_mul(out=rz, in0=rz, scalar1=0.25)
            nc.vector.tensor_tensor(
                out=es.rearrange("p (h j) -> p h j", h=4),
                in0=es.rearrange("p (h j) -> p h j", h=4),
                in1=rz.unsqueeze(2).to_broadcast((128, 4, 32)), op=OP.mult)
            nc.vector.tensor_reduce(
                out=ap8[c], in_=es.rearrange("p (h j) -> p j h", h=4),
                axis=AX.X, op=OP.add)
            nc.vector.tensor_tensor(out=em, in0=em, in1=rz, op=OP.mult)
            nc.vector.tensor_reduce(out=fm[c], in_=em, axis=AX.X, op=OP.add)
            _cap(f"ap8_{c}", ap8[c])
            _cap(f"fm_{c}", fm[c])
            # --- candidate spread for this chunk
            nc.vector.tensor_tensor(
                out=sprd[c][:, 0:256].rearrange("p (i j) -> p i j", i=8),
                in0=ap8[c].unsqueeze(1).to_broadcast((128, 8, 32)),
                in1=ax('spreadm').rearrange("p (i j) -> p i j", i=8),
                op=OP.mult)
            nc.vector.tensor_tensor(
                out=sprd[c][:, 256:264],
                in0=fm[c].to_broadcast((128, 8)),
                in1=ax('oneh8'), op=OP.mult)
        for dc in range(3):
            _cap(f"featsT{dc}", featsT[dc][0])
        for h in range(H):
            _cap(f"qT{h}", qT[h][0])
            _cap(f"kT{h}", kT[h][0])
        for c in range(4):
            _cap(f"g{c}", g[c])

        # ---- stage E: sentence-major candidates via block-ones matmuls
        candvd = singles.tile([128, NCAND], F32)
        nc.vector.tensor_copy(out=candvd[:, :], in_=ax('candc'))
        psm = ps.tile([128, 264], F32, tag="ps", name="psm")
        for c in range(4):
            nc.tensor.matmul(psm, lhsT=ax('blk128')[:, 128*c:128*c+128],
                             rhs=sprd[c], start=(c == 0), stop=(c == 3))
        sm64 = work.tile([128, 264], F32, tag="sm64", name="sm64")
        nc.scalar.activation(out=sm64, in_=psm, func=AF.Copy)
        blk = sm64[:, 0:256].rearrange(
            "(x ss) (i cc) -> x ss i cc", ss=4, i=8)
        fil = sm64[:, 256:264].rearrange("(x ss) f -> x ss f", ss=4)
        dv = candvd.rearrange("(x ss) n -> x ss n", ss=4)
        for ss in range(4):
            eng = nc.sync if ss % 2 == 0 else nc.gpsimd
            eng.dma_start(
                out=dv[:, ss:ss+1, 0:64].rearrange(
                    "x ss (i j) -> x ss i j", i=8),
                in_=blk[:, ss:ss+1, :, 8*ss:8*ss+8])
            eng.dma_start(out=dv[:, ss:ss+1, 64:72],
                          in_=fil[:, ss:ss+1, :])
        candv = candvd[0:64, :]
        _cap("candv", candvd)

        # ---- stage F: exact kth-largest via weighted all-pairs rank
        # 2-way split: partition p holds sentence p%64, b-half p//64
        NH = NCAND // 2
        candvh = singles.tile([128, NH], F32, name="candvh")
        nc.vector.tensor_copy(out=candvh[0:64, :], in_=candvd[0:64, 0:NH])
        nc.vector.tensor_copy(out=candvh[64:128, :],
                              in_=candvd[64:128, NH:NCAND])
        pc = singles.tile([128, NCAND*NH], F32)
        nc.vector.tensor_tensor(
            out=pc.rearrange("s (a b) -> s a b", a=NCAND),
            in0=candvh.unsqueeze(1).to_broadcast((128, NCAND, NH)),
            in1=candvd.unsqueeze(2).to_broadcast((128, NCAND, NH)),
            op=OP.is_ge)
        nc.vector.tensor_tensor(
            out=pc.rearrange("s (a b) -> s a b", a=NCAND),
            in0=pc.rearrange("s (a b) -> s a b", a=NCAND),
            in1=ax('candwh').unsqueeze(1).to_broadcast((128, NCAND, NH)),
            op=OP.mult)
        rkh = work.tile([128, NCAND], F32, tag="rkh")
        nc.vector.tensor_reduce(
            out=rkh, in_=pc.rearrange("s (a b) -> s a b", a=NCAND),
            axis=AX.X, op=OP.add)
        prk = ps.tile([64, NCAND], F32, tag="ps", name="prk")
        nc.tensor.matmul(prk, lhsT=ax('sumh'), rhs=rkh,
                         start=True, stop=True)
        rk = work.tile([BS, NCAND], F32, tag="rk")
        nc.scalar.activation(out=rk, in_=prk, func=AF.Copy)
        t1 = work.tile([BS, NCAND], F32, tag="t1")
        nc.vector.scalar_tensor_tensor(
            out=t1, in0=rk, scalar=float(KTOP), in1=candv,
            op0=OP.is_ge, op1=OP.mult)
        nc.vector.tensor_tensor(out=t1, in0=t1, in1=ax('cmask'), op=OP.mult)
        tstar = work.tile([BS, 1], F32, tag="tstar")
        nc.vector.tensor_reduce(out=tstar, in_=t1, axis=AX.X, op=OP.max)
        _cap("rk", rk)
        _cap("tstar", tstar)

        # ---- stage G: W_final / denom
        c1 = work.tile([BS, 64], F32, tag="c1")
        nc.vector.tensor_scalar(out=c1, in0=candv[:, 0:64],
                                scalar1=tstar[:, 0:1], scalar2=None,
                                op0=OP.is_ge)
        wfin = singles.tile([BS, 64], F32)
        nc.vector.tensor_tensor(
            out=wfin.rearrange("s (i j) -> s i j", i=8),
            in0=c1.rearrange("s (i j) -> s i j", i=8),
            in1=c1.rearrange("s (j i) -> s i j", j=8), op=OP.add)
        nc.vector.tensor_tensor(out=wfin, in0=wfin, in1=candv[:, 0:64],
                                op=OP.mult)
        nc.vector.tensor_tensor(out=wfin, in0=wfin, in1=ax('mv'), op=OP.mult)
        nc.vector.tensor_tensor(out=wfin, in0=wfin, in1=ax('dg64'), op=OP.add)
        rsum = work.tile([BS, 8], F32, tag="rsum")
        nc.vector.tensor_reduce(out=rsum, in_=wfin.rearrange(
            "s (i j) -> s i j", i=8), axis=AX.X, op=OP.add)
        cf = work.tile([BS, 8], F32, tag="cf")
        nc.vector.tensor_scalar(out=cf, in0=candv[:, 64:72],
                                scalar1=tstar[:, 0:1], scalar2=None,
                                op0=OP.is_ge)
        c1l = work.tile([BS, 1], F32, tag="c1l")
        nc.vector.tensor_scalar(out=c1l, in0=ax('linv'),
                                scalar1=tstar[:, 0:1], scalar2=None,
                                op0=OP.is_ge)
        nc.vector.scalar_tensor_tensor(
            out=cf, in0=cf, scalar=c1l[:, 0:1], in1=candv[:, 64:72],
            op0=OP.add, op1=OP.mult)
        nc.vector.tensor_tensor(out=cf, in0=cf, in1=ax('mfrow'), op=OP.mult)
        den = work.tile([BS, 8], F32, tag="den")
        nc.vector.scalar_tensor_tensor(
            out=den, in0=cf, scalar=ax('lennv64'), in1=rsum,
            op0=OP.mult, op1=OP.add)
        nc.vector.tensor_scalar_add(out=den, in0=den, scalar1=1.0)
        rd = work.tile([BS, 8], F32, tag="rd")
        nc.vector.reciprocal(out=rd, in_=den)
        _cap("wfin", wfin)
        _cap("rd", rd)

        # ---- stage H: block-diag aggregation + epilogue
        for mt in range(2):
            pagg = ps.tile([128, D], F32, tag="ps", name=f"pagg{mt}")
            for kc in range(2):
                # wl [128, 4] via partition-spread matmul + pick
                pwl = ps.tile([128, 32], F32, tag="ps", name=f"pwl{mt}{kc}")
                nc.tensor.matmul(
                    pwl, lhsT=ax('wlmap')[:, 128*(2*mt+kc):128*(2*mt+kc)+128],
                    rhs=wfin[:, 32:64], start=True, stop=True)
                wlt = work.tile([128, 32], F32, tag="wlt")
                nc.vector.tensor_tensor(out=wlt, in0=pwl, in1=ax('wlpick'),
                                        op=OP.mult)
                wl = work.tile([128, 4], F32, tag="wl")
                nc.vector.tensor_reduce(
                    out=wl, in_=wlt.rearrange("p (ct j) -> p ct j", ct=4),
                    axis=AX.X, op=OP.add)
                wblk = work.tile([128, 128], F32, tag="wblk")
                hm = ax('hm0') if kc == 0 else ax('hm1')
                nc.vector.tensor_tensor(
                    out=wblk.rearrange("p (blk ct) -> p blk ct", blk=32),
                    in0=wl.unsqueeze(1).to_broadcast((128, 32, 4)),
                    in1=hm.rearrange("p (blk ct) -> p blk ct", blk=32),
                    op=OP.mult)
                nc.tensor.matmul(pagg, lhsT=wblk, rhs=g[2*mt+kc],
                                 start=(kc == 0), stop=(kc == 1))
            # rdp [128, 1] via partition-spread matmul + pick
            prd = ps.tile([128, 4], F32, tag="ps", name=f"prd{mt}")
            nc.tensor.matmul(
                prd, lhsT=ax('rdmap')[:, 128*mt:128*mt+128],
                rhs=rd[:, 4:8], start=True, stop=True)
            rdt = work.tile([128, 4], F32, tag="rdt")
            nc.vector.tensor_tensor(out=rdt, in0=prd, in1=ax('rdpick'),
                                    op=OP.mult)
            rdp = work.tile([128, 1], F32, tag="rdp")
            nc.vector.tensor_reduce(out=rdp, in_=rdt, axis=AX.X, op=OP.add)
            osb = work.tile([128, D], F32, tag="osb")
            nc.scalar.activation(out=osb, in_=pagg, func=AF.Relu,
                                 scale=rdp[:, 0:1])
            nc.sync.dma_start(out=out_d[128*mt:128*mt+128, :], in_=osb)


# ------------------------------------------------------------------- runner
_NC_CACHE = {}


def _get_nc():
    if 'nc' not in _NC_CACHE:
        _NC_CACHE['nc'] = build_nc()
    return _NC_CACHE['nc']


def _in_maps(inputs, cores):
    WqT = np.ascontiguousarray(np.asarray(inputs['Wq'], np.float32).T)
    WkT = np.ascontiguousarray(np.asarray(inputs['Wk'], np.float32).T)
    WgT = np.ascontiguousarray(np.asarray(inputs['Wg'], np.float32).T)
    maps = []
    for ci in cores:
        m = {k: np.ascontiguousarray(v) for k, v in ci.items()}
        m['WqT'], m['WkT'], m['WgT'] = WqT, WkT, WgT
        maps.append(m)
    return maps


def kernel(**inputs):
    cores, post = _host_prep(inputs)
    in_maps = _in_maps(inputs, cores)
    nc = _get_nc()
    res = run_bass_kernel_spmd(nc, in_maps, core_ids=list(range(NCORE)))
    outs = [np.asarray(r['out']) for r in res.results]

    co = np.asarray(inputs['clause_output'])
    lens = post['lens']
    result = np.empty((NC, D), np.float32)
    orow, ocid = post['out_row'], post['out_cidx']
    for n in range(NC):
        b, ct = orow[n], ocid[n]
        if lens[b] > 1:
            c, s = b // BS, b % BS
            result[n] = outs[c][128*(s // 32) + 4*(s % 32) + ct]
        else:
            result[n] = co[n]
    return result.astype(np.asarray(inputs['clause_output']).dtype)


# revision 30
# speedup vs baseline: 13.8132x; 13.8132x over previous
"""Trainium2 Bass kernel for nn_AA_encoder (gnn_message_passing).

Data-parallel over sentences B=512 -> 64 per core on 8 NeuronCores.
Sparse-graph decomposition: each sentence has <=8 real node slots
(4 aspect + 4 clause); packed 8x8 attention per sentence plus analytic
reconstruction of the full 48x48 softmax / top-k / GCN.

Self-contained: host-side index preprocessing (numpy) + one SPMD Bass
program executed via run_bass_kernel_spmd on cores 0-7.
"""
import sys
import numpy as np

sys.path.insert(0, '/opt/trn_rl_repo')

import concourse.bass as bass
import concourse.bacc as bacc
import concourse.tile as tile
from concourse import mybir
from concourse.bass_utils import run_bass_kernel_spmd

B, L, T, D = 512, 48, 128, 300
NA = NC = 2048
H, KTOP = 4, 96
DK = D // H
NCORE = 8
BS = B // NCORE          # 64 sentences per core
A = C = 4                # aspects / clauses per sentence
R = A + C                # 8 real rows per sentence
NCAND = 80
F32 = mybir.dt.float32
AX = mybir.AxisListType
OP = mybir.AluOpType
AF = mybir.ActivationFunctionType

# ---------------------------------------------------------------- aux layout
# one packed [128, NAUX] constant tensor -> single DMA
_AUX_SPECS = [
    ('maskT', 128, 256), ('vcm', 128, 512), ('lennv', 128, 4),
    ('hm0', 128, 128), ('hm1', 128, 128), ('spreadm', 128, 256),
    ('oneh8', 128, 8), ('blk128', 128, 512),
    ('candwh', 128, 40), ('sumh', 128, 64),
    ('candw', 64, NCAND), ('candc', 128, NCAND), ('cmask', 64, NCAND),
    ('mv', 64, 64), ('dg64', 64, 64), ('mfrow', 64, R),
    ('linv', 64, 1), ('lennv64', 64, 1),
    ('wlmap', 64, 512), ('wlpick', 128, 32),
    ('rdmap', 64, 256), ('rdpick', 128, 4),
]
_AUX_OFF = {}
_off = 0
for _nm, _rows, _cols in _AUX_SPECS:
    _AUX_OFF[_nm] = (_rows, _off, _cols)
    _off += _cols
NAUX = _off

# layout constants: p = 32*qq + 8*ss + ii
_SPREADM = np.zeros((128, 256), np.float32)
_ONEH8 = np.zeros((128, 8), np.float32)
_BLK128 = np.zeros((128, 512), np.float32)
_SUMH = np.zeros((128, 64), np.float32)
_HM = np.zeros((2, 128, 128), np.float32)
_WLMAP = np.zeros((64, 512), np.float32)
_WLPICK = np.zeros((128, 32), np.float32)
_RDMAP = np.zeros((64, 256), np.float32)
_RDPICK = np.zeros((128, 4), np.float32)
for _p in range(128):
    _qq, _ss, _ii = _p // 32, (_p % 32) // 8, _p % 8
    _ONEH8[_p, _ii] = 1.0
    for _c4 in range(4):
        _s16g = 16*_c4 + 4*_qq + _ss
        _BLK128[_p, 128*_c4 + _s16g] = 1.0
        _BLK128[_p, 128*_c4 + 64 + _s16g] = 1.0
    _SUMH[_p, _p % 64] = 1.0
    for _j in range(8):
        _SPREADM[_p, 32*_ii + 8*_ss + _j] = 1.0
    _i16, _j8 = _p // 8, _p % 8
    for _kc in range(2):
        if _i16 < 16:
            _HM[_kc, _p, 64*_kc + 4*_i16: 64*_kc + 4*_i16 + 4] = 1.0
    for _mt in range(2):
        for _kc in range(2):
            _s = 32*_mt + 16*_kc + _i16
            _WLMAP[_s, 128*(2*_mt + _kc) + _p] = 1.0
    _WLPICK[_p, _j8::8] = 1.0          # [j == p%8] in each ct group
    for _mt in range(2):
        _RDMAP[32*_mt + _p // 4, 128*_mt + _p] = 1.0
    _RDPICK[_p, _p % 4] = 1.0


# ----------------------------------------------------------------- host prep
def _host_prep(inputs):
    ie = np.ascontiguousarray(np.asarray(inputs['input_embed'], np.float32))
    co = np.ascontiguousarray(np.asarray(inputs['clause_output'], np.float32))
    bm = np.asarray(inputs['batch_aa_mask'], np.float32)
    gl = np.asarray(inputs['aa_graph_length']).astype(np.int64)
    mAA = np.asarray(inputs['map_AA']).astype(np.int64)
    mAAi = np.asarray(inputs['map_AA_idx']).astype(np.int64)
    mAS = np.asarray(inputs['map_AS']).astype(np.int64)
    mASi = np.asarray(inputs['map_AS_idx']).astype(np.int64)
    for bname in ('bq', 'bk', 'bg'):
        assert np.all(np.asarray(inputs[bname]) == 0), f"{bname} != 0 unsupported"

    asp_rows = [[] for _ in range(B)]
    for n in range(NA):
        asp_rows[mAA[n]].append(n)
    cls_rows = [[] for _ in range(B)]
    for n in range(NC):
        cls_rows[mAS[n]].append(n)
    assert max(len(x) for x in asp_rows) <= A
    assert max(len(x) for x in cls_rows) <= C

    slots = np.full((B, R), -1, np.int64)
    maskTg = np.zeros((B, T, A), np.float32)
    clsg = np.zeros((B, C, D), np.float32)
    out_row = np.full((NC,), -1, np.int64)
    out_cidx = np.full((NC,), -1, np.int64)
    for b in range(B):
        cslots = set()
        for j, n in enumerate(cls_rows[b]):
            s = mASi[n]
            assert s not in cslots
            cslots.add(s)
            slots[b, A + j] = s
            clsg[b, j] = co[n]
            out_row[n] = b
            out_cidx[n] = j
        aslots = set()
        for j, n in enumerate(asp_rows[b]):
            s = mAAi[n]
            assert s not in aslots
            aslots.add(s)
            if s in cslots:
                continue
            slots[b, j] = s
            maskTg[b, :, j] = bm[n]

    lens = gl
    vr = (slots >= 0)
    vc = vr & (slots < lens[:, None])
    nv = vc.sum(1)

    cores = []
    for c in range(NCORE):
        s0 = c * BS
        sl_vc = vc[s0:s0+BS]
        nvc = nv[s0:s0+BS]
        ln = lens[s0:s0+BS]

        vcm = np.zeros((4, 128, 128), np.float32)
        lennv = np.zeros((4, 128, 1), np.float32)
        for s in range(BS):
            ch, i16 = s // 16, s % 16
            qq, ss = i16 // 4, i16 % 4
            vcj = sl_vc[s].astype(np.float32)
            for ii in range(R):
                p = 32*qq + 8*ss + ii
                lennv[ch, p, 0] = ln[s] - nvc[s]
                for h in range(H):
                    vcm[ch, p, 32*h + 8*ss: 32*h + 8*ss + 8] = vcj

        candw = np.zeros((BS, NCAND), np.float32)
        candc = np.zeros((BS, NCAND), np.float32)
        cmask = np.zeros((BS, NCAND), np.float32)
        mv = np.zeros((BS, 64), np.float32)
        dg64 = np.zeros((BS, 64), np.float32)
        mfrow = np.zeros((BS, R), np.float32)
        linv = (1.0 / ln[:, None]).astype(np.float32)
        lennv64 = (ln - nvc)[:, None].astype(np.float32)
        for s in range(BS):
            l, n_v = int(ln[s]), int(nvc[s])
            vcs = sl_vc[s]
            for i in range(R):
                for j in range(R):
                    if i != j and vcs[i] and vcs[j]:
                        candw[s, 8*i+j] = 1.0
                        cmask[s, 8*i+j] = 1.0
                        mv[s, 8*i+j] = 1.0
                if vcs[i]:
                    candw[s, 64+i] = l - n_v
                    cmask[s, 64+i] = 1.0 if l > n_v else 0.0
                    dg64[s, 8*i+i] = 1.0
                    mfrow[s, i] = 1.0
            candc[s, 72] = 1.0; candw[s, 72] = l; cmask[s, 72] = 1.0
            candc[s, 73] = 1.0/l
            candw[s, 73] = (l - n_v) * (l - 1)
            cmask[s, 73] = 1.0 if (l - n_v) > 0 and l > 1 else 0.0
            used = l + (l-n_v)*(l-1) + sum(
                (n_v-1) + (l-n_v) for i in range(R) if vcs[i])
            candw[s, 74] = L*L - used
            cmask[s, 74] = 1.0

        clsc = clsg[s0:s0+BS].reshape(BS*C, D)

        aux = np.zeros((128, NAUX), np.float32)
        parts = dict(
            maskT=np.ascontiguousarray(
                maskTg[s0:s0+BS].transpose(1, 0, 2).reshape(T, BS*A)),
            vcm=vcm.transpose(1, 0, 2).reshape(128, 512),
            lennv=lennv.transpose(1, 0, 2).reshape(128, 4),
            hm0=_HM[0], hm1=_HM[1], spreadm=_SPREADM, oneh8=_ONEH8,
            blk128=_BLK128, sumh=_SUMH,
            candwh=np.concatenate([candw[:, 0:40], candw[:, 40:80]], axis=0),
            candw=candw, candc=np.vstack([candc, candc]),
            cmask=cmask,
            mv=mv, dg64=dg64, mfrow=mfrow, linv=linv, lennv64=lennv64,
            wlmap=_WLMAP, wlpick=_WLPICK, rdmap=_RDMAP, rdpick=_RDPICK,
        )
        for nm, arr in parts.items():
            rows, off, cols = _AUX_OFF[nm]
            assert arr.shape == (rows, cols), (nm, arr.shape)
            aux[:rows, off:off+cols] = arr

        cores.append(dict(
            emb=ie[s0:s0+BS],
            clsT=np.ascontiguousarray(clsc.T),             # [300, 256]
            aux=aux,
        ))
    post = dict(out_row=out_row, out_cidx=out_cidx, lens=lens)
    return cores, post


# -------------------------------------------------------------- bass program
def _make_nc():
    nc = bacc.Bacc("TRN2", target_bir_lowering=False, debug=False,
                   enable_asserts=False, num_devices=NCORE)
    io = {}
    specs = dict(
        emb=(BS, T, D), clsT=(D, BS*C), aux=(128, NAUX),
        WqT=(D, D), WkT=(D, D), WgT=(D, D),
    )
    for k, shp in specs.items():
        io[k] = nc.dram_tensor(k, list(shp), F32, kind="ExternalInput")
    out_d = nc.dram_tensor("out", [2*128, D], F32, kind="ExternalOutput")
    return nc, io, out_d


def build_nc():
    nc, io, out_d = _make_nc()
    with tile.TileContext(nc) as tc:
        _build_body(nc, tc, io, out_d)
    nc.finalize()
    return nc


def build_nc_debug(names):
    nc, io, out_d = _make_nc()
    dbg = {'names': set(names), 'dumps': []}
    with tile.TileContext(nc) as tc:
        _build_body(nc, tc, io, out_d, dbg=dbg)
        for nm, t in dbg['dumps']:
            od = nc.dram_tensor(f"dbg_{nm}", list(t.shape), F32,
                                kind="ExternalOutput")
            nc.sync.dma_start(out=od[tuple(slice(None) for _ in t.shape)],
                              in_=t)
    nc.finalize()
    return nc


def _build_body(nc, tc, io, out_d, dbg=None):
    import contextlib

    def _cap(name, t):
        if dbg is not None and name in dbg['names']:
            dbg['dumps'].append((name, t))

    ctx = contextlib.ExitStack()
    with ctx:
        singles = ctx.enter_context(tc.tile_pool(name="singles", bufs=1))
        embp = ctx.enter_context(tc.tile_pool(name="embp", bufs=8))
        work = ctx.enter_context(tc.tile_pool(name="work", bufs=4))
        ps = ctx.enter_context(tc.tile_pool(name="ps", bufs=8, space="PSUM"))

        # ---- resident constants: ONE aux DMA + 3 weight DMAs
        aux = singles.tile([128, NAUX], F32)
        nc.sync.dma_start(out=aux[:, :], in_=io['aux'][:, :])

        def ax(nm):
            rows, off, cols = _AUX_OFF[nm]
            return aux[0:rows, off:off+cols]

        wq = singles.tile([100, 3*D], F32, tag="w0", name="wq")
        wk = singles.tile([100, 3*D], F32, tag="w1", name="wk")
        wg = singles.tile([100, 3*D], F32, tag="w2", name="wg")
        for wtile, wio in ((wq, io['WqT']), (wk, io['WkT']), (wg, io['WgT'])):
            nc.sync.dma_start(
                out=wtile.rearrange("r (c d) -> r c d", c=3),
                in_=wio.rearrange("(c r) d -> r c d", c=3))

        maskT = ax('maskT')
        clsS = [singles.tile([100, 256], F32, tag=f"clsS{dc}",
                             name=f"clsS{dc}") for dc in range(3)]
        for dc in range(3):
            nc.sync.dma_start(out=clsS[dc][:, :],
                              in_=io['clsT'][100*dc:100*dc+100, :])

        # ---- fused per-chunk pipeline: aa -> qk/g -> scores -> softmax
        featsT = [[singles.tile([100, 128], F32, tag=f"fT{dc}_{c}",
                                 name=f"fT{dc}_{c}") for c in range(4)]
                  for dc in range(3)]
        g = [singles.tile([128, D], F32, tag=f"g{c}", name=f"g{c}")
             for c in range(4)]
        qT = [[singles.tile([75, 128], F32, tag=f"qT{h}_{c}",
                             name=f"qT{h}_{c}") for c in range(4)]
              for h in range(H)]
        kT = [[singles.tile([75, 128], F32, tag=f"kT{h}_{c}",
                            name=f"kT{h}_{c}") for c in range(4)]
              for h in range(H)]
        ap8 = [work.tile([128, 32], F32, tag=f"ap8{c}", name=f"ap8{c}")
               for c in range(4)]
        fm = [work.tile([128, 1], F32, tag=f"fm{c}", name=f"fm{c}")
              for c in range(4)]
        sprd = [work.tile([128, 264], F32, tag=f"sprd{c}", name=f"sprd{c}")
                for c in range(4)]
        emb_r = io['emb']  # [64, 128, 300]
        sc = float(1.0 / np.sqrt(DK))
        for c in range(4):
            # --- aa-embed (transposed) via 4-sentence emb bursts
            for g4 in range(4):
                esb = embp.tile([T, 4*D], F32, tag="emb")
                nc.sync.dma_start(
                    out=esb.rearrange("t (s d) -> t s d", s=4),
                    in_=emb_r[16*c+4*g4:16*c+4*g4+4].rearrange(
                        "s t d -> t s d"))
                pst = [ps.tile([100, 32], F32, tag="ps",
                               name=f"pst{c}_{g4}_{i}") for i in range(3)]
                for sl in range(4):
                    s = 16*c + 4*g4 + sl
                    for dc in range(3):
                        nc.tensor.matmul(
                            pst[dc][:, 8*sl:8*sl+4],
                            lhsT=esb[:, 300*sl+100*dc:300*sl+100*dc+100],
                            rhs=maskT[:, 4*s:4*s+4],
                            start=True, stop=True, skip_group_check=True)
                for dc in range(3):
                    nc.scalar.activation(
                        out=featsT[dc][c][:, 32*g4:32*g4+32],
                        in_=pst[dc], func=AF.Copy)
            # clause columns (overwrite garbage at p=8i+4..8i+8)
            for dc in range(3):
                nc.vector.tensor_copy(
                    out=featsT[dc][c].rearrange(
                        "d (i r) -> d i r", i=16)[:, :, 4:8],
                    in_=clsS[dc][:, 64*c:64*c+64].rearrange(
                        "d (i r) -> d i r", r=4))
        for c in range(4):
            # --- q/k projections (head-major [75, 128]) and g row-major
            for h in range(H):
                pq = ps.tile([75, 128], F32, tag="ps", name=f"pq{c}_{h}")
                for k in range(3):
                    nc.tensor.matmul(
                        pq, lhsT=wq[:, k*D+75*h:k*D+75*h+75],
                        rhs=featsT[k][c],
                        start=(k == 0), stop=(k == 2))
                nc.scalar.activation(out=qT[h][c][:, :],
                                     in_=pq, func=AF.Copy, scale=sc)
                pk = ps.tile([75, 128], F32, tag="ps", name=f"pk{c}_{h}")
                for k in range(3):
                    nc.tensor.matmul(
                        pk, lhsT=wk[:, k*D+75*h:k*D+75*h+75],
                        rhs=featsT[k][c],
                        start=(k == 0), stop=(k == 2))
                nc.scalar.activation(out=kT[h][c][:, :],
                                     in_=pk, func=AF.Copy)
            pg = ps.tile([128, D], F32, tag="ps", name=f"pg{c}")
            for k in range(3):
                nc.tensor.matmul(
                    pg, lhsT=featsT[k][c],
                    rhs=wg[:, k*D:k*D+D], start=(k == 0), stop=(k == 2))
            nc.scalar.activation(out=g[c], in_=pg, func=AF.Copy)
        for c in range(4):
            # --- scores
            vcm_c = ax('vcm')[:, 128*c:128*c+128]
            lennv_c = ax('lennv')[:, c:c+1]
            psc = ps.tile([128, 128], F32, tag="ps", name=f"psc{c}")
            for qq in range(4):
                q0 = 32*qq
                for h in range(H):
                    nc.tensor.matmul(
                        psc[32*qq:32*qq+32, 32*h:32*h+32],
                        lhsT=qT[h][c][:, q0:q0+32],
                        rhs=kT[h][c][:, q0:q0+32],
                        start=True, stop=True, skip_group_check=True,
                        tile_position=(0, 32*qq))
            # --- masked softmax reconstruction
            smb = work.tile([128, 128], F32, tag="smb")
            nc.vector.tensor_tensor(out=smb, in0=psc, in1=vcm_c, op=OP.mult)
            m4 = work.tile([128, 4], F32, tag="m4")
            nc.vector.tensor_reduce(out=m4, in_=smb.rearrange(
                "p (h j) -> p h j", h=4), axis=AX.X, op=OP.max)
            nc.vector.tensor_scalar_max(out=m4, in0=m4, scalar1=0.0)
            es = work.tile([128, 128], F32, tag="es")
            nc.vector.tensor_tensor(
                out=es.rearrange("p (h j) -> p h j", h=4),
                in0=smb.rearrange("p (h j) -> p h j", h=4),
                in1=m4.unsqueeze(2).to_broadcast((128, 4, 32)),
                op=OP.subtract)
            nc.scalar.activation(out=es, in_=es, func=AF.Exp)
            nc.vector.tensor_tensor(out=es, in0=es, in1=vcm_c, op=OP.mult)
            z4 = work.tile([128, 4], F32, tag="z4")
            nc.vector.tensor_reduce(out=z4, in_=es.rearrange(
                "p (h j) -> p h j", h=4), axis=AX.X, op=OP.add)
            em = work.tile([128, 4], F32, tag="em")
            nc.scalar.activation(out=em, in_=m4, func=AF.Exp, scale=-1.0)
            nc.vector.scalar_tensor_tensor(
                out=z4, in0=em, scalar=lennv_c, in1=z4,
                op0=OP.mult, op1=OP.add)
            rz = work.tile([128, 4], F32, tag="rz")
            nc.vector.reciprocal(out=rz, in_=z4)
            nc.vector.tensor_scalar_mul(out=rz, in0=rz, scalar1=0.25)
            nc.vector.tensor_tensor(
                out=es.rearrange("p (h j) -> p h j", h=4),
                in0=es.rearrange("p (h j) -> p h j", h=4),
                in1=rz.unsqueeze(2).to_broadcast((128, 4, 32)), op=OP.mult)
            nc.vector.tensor_reduce(
                out=ap8[c], in_=es.rearrange("p (h j) -> p j h", h=4),
                axis=AX.X, op=OP.add)
            nc.vector.tensor_tensor(out=em, in0=em, in1=rz, op=OP.mult)
            nc.vector.tensor_reduce(out=fm[c], in_=em, axis=AX.X, op=OP.add)
            _cap(f"ap8_{c}", ap8[c])
            _cap(f"fm_{c}", fm[c])
            # --- candidate spread for this chunk
            nc.vector.tensor_tensor(
                out=sprd[c][:, 0:256].rearrange("p (i j) -> p i j", i=8),
                in0=ap8[c].unsqueeze(1).to_broadcast((128, 8, 32)),
                in1=ax('spreadm').rearrange("p (i j) -> p i j", i=8),
                op=OP.mult)
            nc.vector.tensor_tensor(
                out=sprd[c][:, 256:264],
                in0=fm[c].to_broadcast((128, 8)),
                in1=ax('oneh8'), op=OP.mult)
        for dc in range(3):
            _cap(f"featsT{dc}", featsT[dc][0])
        for h in range(H):
            _cap(f"qT{h}", qT[h][0])
            _cap(f"kT{h}", kT[h][0])
        for c in range(4):
            _cap(f"g{c}", g[c])

        # ---- stage E: sentence-major candidates via block-ones matmuls
        candvd = singles.tile([128, NCAND], F32)
        nc.vector.tensor_copy(out=candvd[:, :], in_=ax('candc'))
        psm = ps.tile([128, 264], F32, tag="ps", name="psm")
        for c in range(4):
            nc.tensor.matmul(psm, lhsT=ax('blk128')[:, 128*c:128*c+128],
                             rhs=sprd[c], start=(c == 0), stop=(c == 3))
        sm64 = work.tile([128, 264], F32, tag="sm64", name="sm64")
        nc.scalar.activation(out=sm64, in_=psm, func=AF.Copy)
        blk = sm64[:, 0:256].rearrange(
            "(x ss) (i cc) -> x ss i cc", ss=4, i=8)
        fil = sm64[:, 256:264].rearrange("(x ss) f -> x ss f", ss=4)
        dv = candvd.rearrange("(x ss) n -> x ss n", ss=4)
        for ss in range(4):
            eng = nc.sync if ss % 2 == 0 else nc.gpsimd
            eng.dma_start(
                out=dv[:, ss:ss+1, 0:64].rearrange(
                    "x ss (i j) -> x ss i j", i=8),
                in_=blk[:, ss:ss+1, :, 8*ss:8*ss+8])
            eng.dma_start(out=dv[:, ss:ss+1, 64:72],
                          in_=fil[:, ss:ss+1, :])
        candv = candvd[0:64, :]
        _cap("candv", candvd)

        # ---- stage F: exact kth-largest via weighted all-pairs rank
        # 2-way split: partition p holds sentence p%64, b-half p//64
        NH = NCAND // 2
        candvh = singles.tile([128, NH], F32, name="candvh")
        nc.vector.tensor_copy(out=candvh[0:64, :], in_=candvd[0:64, 0:NH])
        nc.vector.tensor_copy(out=candvh[64:128, :],
                              in_=candvd[64:128, NH:NCAND])
        pc = singles.tile([128, NCAND*NH], F32)
        nc.vector.tensor_tensor(
            out=pc.rearrange("s (a b) -> s a b", a=NCAND),
            in0=candvh.unsqueeze(1).to_broadcast((128, NCAND, NH)),
            in1=candvd.unsqueeze(2).to_broadcast((128, NCAND, NH)),
            op=OP.is_ge)
        nc.vector.tensor_tensor(
            out=pc.rearrange("s (a b) -> s a b", a=NCAND),
            in0=pc.rearrange("s (a b) -> s a b", a=NCAND),
            in1=ax('candwh').unsqueeze(1).to_broadcast((128, NCAND, NH)),
            op=OP.mult)
        rkh = work.tile([128, NCAND], F32, tag="rkh")
        nc.vector.tensor_reduce(
            out=rkh, in_=pc.rearrange("s (a b) -> s a b", a=NCAND),
            axis=AX.X, op=OP.add)
        prk = ps.tile([64, NCAND], F32, tag="ps", name="prk")
        nc.tensor.matmul(prk, lhsT=ax('sumh'), rhs=rkh,
                         start=True, stop=True)
        rk = work.tile([BS, NCAND], F32, tag="rk")
        nc.scalar.activation(out=rk, in_=prk, func=AF.Copy)
        t1 = work.tile([BS, NCAND], F32, tag="t1")
        nc.vector.scalar_tensor_tensor(
            out=t1, in0=rk, scalar=float(KTOP), in1=candv,
            op0=OP.is_ge, op1=OP.mult)
        nc.vector.tensor_tensor(out=t1, in0=t1, in1=ax('cmask'), op=OP.mult)
        tstar = work.tile([BS, 1], F32, tag="tstar")
        nc.vector.tensor_reduce(out=tstar, in_=t1, axis=AX.X, op=OP.max)
        _cap("rk", rk)
        _cap("tstar", tstar)

        # ---- stage G: W_final / denom
        c1 = work.tile([BS, 64], F32, tag="c1")
        nc.vector.tensor_scalar(out=c1, in0=candv[:, 0:64],
                                scalar1=tstar[:, 0:1], scalar2=None,
                                op0=OP.is_ge)
        wfin = singles.tile([BS, 64], F32)
        nc.vector.tensor_tensor(
            out=wfin.rearrange("s (i j) -> s i j", i=8),
            in0=c1.rearrange("s (i j) -> s i j", i=8),
            in1=c1.rearrange("s (j i) -> s i j", j=8), op=OP.add)
        nc.vector.tensor_tensor(out=wfin, in0=wfin, in1=candv[:, 0:64],
                                op=OP.mult)
        nc.vector.tensor_tensor(out=wfin, in0=wfin, in1=ax('mv'), op=OP.mult)
        nc.vector.tensor_tensor(out=wfin, in0=wfin, in1=ax('dg64'), op=OP.add)
        rsum = work.tile([BS, 8], F32, tag="rsum")
        nc.vector.tensor_reduce(out=rsum, in_=wfin.rearrange(
            "s (i j) -> s i j", i=8), axis=AX.X, op=OP.add)
        cf = work.tile([BS, 8], F32, tag="cf")
        nc.vector.tensor_scalar(out=cf, in0=candv[:, 64:72],
                                scalar1=tstar[:, 0:1], scalar2=None,
                                op0=OP.is_ge)
        c1l = work.tile([BS, 1], F32, tag="c1l")
        nc.vector.tensor_scalar(out=c1l, in0=ax('linv'),
                                scalar1=tstar[:, 0:1], scalar2=None,
                                op0=OP.is_ge)
        nc.vector.scalar_tensor_tensor(
            out=cf, in0=cf, scalar=c1l[:, 0:1], in1=candv[:, 64:72],
            op0=OP.add, op1=OP.mult)
        nc.vector.tensor_tensor(out=cf, in0=cf, in1=ax('mfrow'), op=OP.mult)
        den = work.tile([BS, 8], F32, tag="den")
        nc.vector.scalar_tensor_tensor(
            out=den, in0=cf, scalar=ax('lennv64'), in1=rsum,
            op0=OP.mult, op1=OP.add)
        nc.vector.tensor_scalar_add(out=den, in0=den, scalar1=1.0)
        rd = work.tile([BS, 8], F32, tag="rd")
        nc.vector.reciprocal(out=rd, in_=den)
        _cap("wfin", wfin)
        _cap("rd", rd)

        # ---- stage H: block-diag aggregation + epilogue
        for mt in range(2):
            pagg = ps.tile([128, D], F32, tag="ps", name=f"pagg{mt}")
            for kc in range(2):
                # wl [128, 4] via partition-spread matmul + pick
                pwl = ps.tile([128, 32], F32, tag="ps", name=f"pwl{mt}{kc}")
                nc.tensor.matmul(
                    pwl, lhsT=ax('wlmap')[:, 128*(2*mt+kc):128*(2*mt+kc)+128],
                    rhs=wfin[:, 32:64], start=True, stop=True)
                wlt = work.tile([128, 32], F32, tag="wlt")
                nc.vector.tensor_tensor(out=wlt, in0=pwl, in1=ax('wlpick'),
                                        op=OP.mult)
                wl = work.tile([128, 4], F32, tag="wl")
                nc.vector.tensor_reduce(
                    out=wl, in_=wlt.rearrange("p (ct j) -> p ct j", ct=4),
                    axis=AX.X, op=OP.add)
                wblk = work.tile([128, 128], F32, tag="wblk")
                hm = ax('hm0') if kc == 0 else ax('hm1')
                nc.vector.tensor_tensor(
                    out=wblk.rearrange("p (blk ct) -> p blk ct", blk=32),
                    in0=wl.unsqueeze(1).to_broadcast((128, 32, 4)),
                    in1=hm.rearrange("p (blk ct) -> p blk ct", blk=32),
                    op=OP.mult)
                nc.tensor.matmul(pagg, lhsT=wblk, rhs=g[2*mt+kc],
                                 start=(kc == 0), stop=(kc == 1))
            # rdp [128, 1] via partition-spread matmul + pick
            prd = ps.tile([128, 4], F32, tag="ps", name=f"prd{mt}")
            nc.tensor.matmul(
                prd, lhsT=ax('rdmap')[:, 128*mt:128*mt+128],
                rhs=rd[:, 4:8], start=True, stop=True)
            rdt = work.tile([128, 4], F32, tag="rdt")
            nc.vector.tensor_tensor(out=rdt, in0=prd, in1=ax('rdpick'),
                                    op=OP.mult)
            rdp = work.tile([128, 1], F32, tag="rdp")
            nc.vector.tensor_reduce(out=rdp, in_=rdt, axis=AX.X, op=OP.add)
            osb = work.tile([128, D], F32, tag="osb")
            nc.scalar.activation(out=osb, in_=pagg, func=AF.Relu,
                                 scale=rdp[:, 0:1])
            nc.sync.dma_start(out=out_d[128*mt:128*mt+128, :], in_=osb)


# ------------------------------------------------------------------- runner
_NC_CACHE = {}


def _get_nc():
    if 'nc' not in _NC_CACHE:
        _NC_CACHE['nc'] = build_nc()
    return _NC_CACHE['nc']


def _in_maps(inputs, cores):
    WqT = np.ascontiguousarray(np.asarray(inputs['Wq'], np.float32).T)
    WkT = np.ascontiguousarray(np.asarray(inputs['Wk'], np.float32).T)
    WgT = np.ascontiguousarray(np.asarray(inputs['Wg'], np.float32).T)
    maps = []
    for ci in cores:
        m = {k: np.ascontiguousarray(v) for k, v in ci.items()}
        m['WqT'], m['WkT'], m['WgT'] = WqT, WkT, WgT
        maps.append(m)
    return maps


def kernel(**inputs):
    cores, post = _host_prep(inputs)
    in_maps = _in_maps(inputs, cores)
    nc = _get_nc()
    res = run_bass_kernel_spmd(nc, in_maps, core_ids=list(range(NCORE)))
    outs = [np.asarray(r['out']) for r in res.results]

    co = np.asarray(inputs['clause_output'])
    lens = post['lens']
    result = np.empty((NC, D), np.float32)
    orow, ocid = post['out_row'], post['out_cidx']
    for n in range(NC):
        b, ct = orow[n], ocid[n]
        if lens[b] > 1:
            c, s = b // BS, b % BS
            result[n] = outs[c][128*(s // 32) + 4*(s % 32) + ct]
        else:
            result[n] = co[n]
    return result.astype(np.asarray(inputs['clause_output']).dtype)
